# revision 8
# baseline (speedup 1.0000x reference)
"""Channel-attention module kernel for 8 Trainium2 NeuronCores.

reference semantics (B=2, C=128, N=D*H*W=147456):
    q = x.reshape(B, C, N)
    energy = q @ q^T                  # [B, C, C]
    attn = softmax(rowmax(energy) - energy, axis=-1)
          = softmax(-energy, axis=-1)             (rowmax shift is a no-op)
    out = attn @ q
    return x + gamma * out

Sharding: sequence-parallel over N. Core r owns columns
[r*N/8, (r+1)*N/8) of q for both batches. Each core computes a partial
energy (contraction over its local n), the partials are summed across
cores, each core computes softmax and applies attention to its local
columns.

Scheme (v10.2) — no collectives at all. hw-trace findings that shaped it:
  - The CC-engine path (ncfw barrier + AllReduce) doesn't deliver the
    energy sum until ~113us. Instead each core broadcasts its partial
    [C,C] energy per batch to peer r^j's slot j over remote_dma_broadcast
    (XOR-relative dests, 7 one-dest bcasts per batch; slots j>=4 land
    permuted j^2 but the sum is permutation invariant). With no
    collective in the NEFF there is no ncfw cold-start barrier either:
    b0's summed energy is ready ~45us, b1's ~70us.
  - Receiver waits use REGISTER thresholds loaded from an input tensor
    (sim reads 0 -> no scheduler deadlock; HW reads 14), behind a
    TRACKED tensor_copy of the threshold tile (bare reg_load races the
    input DMA). Remote sems are cleared at kernel start on the engine
    that waits on them (sem state persists across NEFF loads).
  - Descriptor PREPS are emitted before phase 1 against pre-initialized
    e_cat tiles (descgen only captures addresses), so the reduce adds
    chain only on the remote sem, not on our own descgen. The TRIGGERS
    are gated on the real e_cat values via tiny gpsimd reads.
  - gpsimd's end-of-body DGE DRAIN costs ~50us once the remote_dma ucode
    library is loaded, and it throttles ALL DMA while it runs. A junk
    standard-lib gpsimd op gated on the last output tile (a) swaps the
    library back in hidden slack and (b) delays the drain past the last
    compute.
  - phase 2 per batch: out = attn_s @ q in bf16 (attn_s = gamma/Z*P + I
    folds the residual; P diagonal exactly 0). b0 uses PE-transposed
    tiles (transpose(k) interleaved with apply(k)); b1's operand xq1
    ([C,n] bf16, just a slice of hi) streams from HBM after all hi/lo
    chunks, where it contends with nothing.
  - hi/lo bf16 split energy as in v9 (two bf16 chains; rel err 1.7e-3
    vs the 2e-2 gate).
"""

import sys

sys.path.insert(0, "/opt/trn_rl_repo")

import numpy as np

B, C = 2, 128
D, H, W = 16, 96, 96
N = D * H * W  # 147456
NCORES = 8
NLOC = N // NCORES  # 18432
T = NLOC // C  # 144 n-tiles of 128 per batch
CHUNK = 2048
NCHUNK = NLOC // CHUNK  # 9
TPC = CHUNK // C  # 16 n-tiles per chunk
OTILE = 512

_compiled = {}


def _log(msg):
    import time as _t
    print(f"[kernel {_t.strftime('%H:%M:%S')}] {msg}", flush=True)


def _build():
    import concourse.bacc as bacc
    import concourse.tile as tile
    import concourse.mybir as mybir

    _log("build start")

    f32 = mybir.dt.float32
    f16 = mybir.dt.float16
    bf16 = mybir.dt.bfloat16
    i32 = mybir.dt.int32
    nc = bacc.Bacc("TRN2", target_bir_lowering=False, debug=False,
                   num_devices=NCORES)

    hi_d = nc.dram_tensor("qhT", [B, C, T * C], bf16, kind="ExternalInput").ap()
    lo_d = nc.dram_tensor("qlT", [B, C, T * C], bf16, kind="ExternalInput").ap()
    xq1_d = nc.dram_tensor("xq1", [C, NLOC], bf16, kind="ExternalInput").ap()
    g_d = nc.dram_tensor("gamma_col", [C, 1], f32, kind="ExternalInput").ap()
    id_d = nc.dram_tensor("ident", [C, C], f32, kind="ExternalInput").ap()
    idb_d = nc.dram_tensor("identb", [C, C], bf16, kind="ExternalInput").ap()
    th_d = nc.dram_tensor("thresh", [1, 2], i32, kind="ExternalInput").ap()
    o_d = nc.dram_tensor("out", [B, C, NLOC], f16, kind="ExternalOutput").ap()

    rsem = [nc.alloc_semaphore(f"rsem_e{b}") for b in range(B)]
    lsem = nc.alloc_semaphore("lsem_gather")

    with tile.TileContext(nc) as tc:
        with (
            tc.tile_pool(name="hring", bufs=NCHUNK + 4) as hp,
            tc.tile_pool(name="lring", bufs=4) as lp,
            tc.tile_pool(name="xb16", bufs=B * NCHUNK) as xbp,
            tc.tile_pool(name="eps", bufs=2, space="PSUM") as eps,
            tc.tile_pool(name="ps", bufs=3, space="PSUM") as ps,
            tc.tile_pool(name="misc", bufs=1) as mp,
            tc.tile_pool(name="ost", bufs=3) as ostp,
        ):
            # --- sem hygiene: clear on the engine that waits, first thing.
            # Receivers' clears run within ~1us of kernel start; the first
            # remote increments arrive only after some sender finishes its
            # b0 partial energy (~40us in), so clears always win the race.
            for b in range(B):
                nc.vector.sem_clear(rsem[b])
            nc.gpsimd.sem_clear(lsem)

            # remote-sem wait threshold in a register (see module docstring)
            th = mp.tile([1, 2], i32, name="th_sb")
            nc.sync.dma_start(th[:], th_d[:])
            th2 = mp.tile([1, 2], i32, name="th2_sb")
            nc.vector.tensor_copy(th2[:], th[:])
            vreg = nc.vector.alloc_register("rth")
            nc.vector.reg_load(vreg, th2[0:1, 1:2])

            ident = mp.tile([C, C], f32, name="ident_sb")
            identb = mp.tile([C, C], bf16, name="identb_sb")
            nc.sync.dma_start(identb[:], idb_d[:])
            nc.sync.dma_start(ident[:], id_d[:])
            # first chunk split so the PE starts during the DMA ramp
            ht0 = hp.tile([C, CHUNK], bf16, name="h_0_0", tag="h")
            nc.sync.dma_start(ht0[:, 0:512], hi_d[0, :, 0:512])
            lt0 = lp.tile([C, CHUNK], bf16, name="l_0_0", tag="l")
            nc.sync.dma_start(lt0[:, 0:512], lo_d[0, :, 0:512])
            nc.sync.dma_start(ht0[:, 512:CHUNK], hi_d[0, :, 512:CHUNK])
            nc.sync.dma_start(lt0[:, 512:CHUNK], lo_d[0, :, 512:CHUNK])
            gcol = mp.tile([C, 1], f32, name="gcol")
            nc.sync.dma_start(gcol[:], g_d[:])

            xb16 = [[xbp.tile([C, CHUNK], bf16, name=f"xb_{b}_{k}", tag="xb")
                     for k in range(NCHUNK)] for b in range(B)]

            # energy gather slots: slot j holds the partial of rank r^j
            # (j>=4 lands permuted j^2 -- sum is invariant)
            eslot = [[mp.tile([C, C], f32, name=f"esl_{b}_{j}")
                      for j in range(1, NCORES)] for b in range(B)]
            # e_cat: local partial per batch. Pre-initialized (copy of
            # ident, value irrelevant) so the descriptor preps emitted
            # before phase 1 have a tracked producer.
            e_cat = []
            for b in range(B):
                t_ = mp.tile([C, C], f32, name=f"e_cat{b}")
                nc.scalar.copy(t_[:], ident[:])
                e_cat.append(t_)
            e_red = [None, None]

            def emit_preps(b):
                # descriptor preps (addresses only; data is read at trigger
                # time by the DMA engines). Emitted before this batch's
                # phase-1 matmuls so descgen runs early on gpsimd.
                for j in range(1, NCORES):
                    rdests = [None] * 8
                    rdests[j] = (0, j)
                    nc.gpsimd.remote_dma_broadcast(
                        eslot[b][j - 1][:], e_cat[b][:], rsem[b], lsem,
                        rdests=rdests)

            hkeep = {}  # live hi chunks of batch 0 (for the transposes)

            def emit_phase1_mms(b):
                e_main = eps.tile([C, C], f32, name=f"em{b}", tag="e")
                e_cross = eps.tile([C, C], f32, name=f"ec{b}", tag="e")
                for k in range(NCHUNK):
                    if b == 0 and k == 0:
                        ht, lt = ht0, lt0
                    else:
                        ht = hp.tile([C, CHUNK], bf16, name=f"h_{b}_{k}",
                                     tag="h")
                        nc.sync.dma_start(
                            ht[:], hi_d[b, :, k * CHUNK:(k + 1) * CHUNK])
                        lt = lp.tile([C, CHUNK], bf16, name=f"l_{b}_{k}",
                                     tag="l")
                        nc.sync.dma_start(
                            lt[:], lo_d[b, :, k * CHUNK:(k + 1) * CHUNK])
                    if b == 0:
                        hkeep[k] = ht
                    if b == 0 and k == 0:
                        # consume the quarter-chunk first so the PE starts
                        # as early as possible during the DMA ramp
                        order = [("hh", j) for j in range(4)] \
                            + [("hl", j) for j in range(4)] \
                            + [p for j in range(4, TPC)
                               for p in (("hh", j), ("hl", j))]
                    else:
                        order = [p for j in range(TPC)
                                 for p in (("hh", j), ("hl", j))]
                    for kind, j in order:
                        t = k * TPC + j
                        hs = ht[:, j * C:(j + 1) * C]
                        if kind == "hh":
                            nc.tensor.matmul(e_main[:], hs, hs,
                                             start=(t == 0), stop=(t == T - 1))
                        else:
                            nc.tensor.matmul(e_cross[:], hs,
                                             lt[:, j * C:(j + 1) * C],
                                             start=(t == 0), stop=(t == T - 1))
                # E_partial = e_main + e_cross + e_cross^T
                ecr = mp.tile([C, C], f32, name=f"ecr{b}")
                nc.vector.tensor_copy(ecr[:], e_cross[:])
                tpc_ps = ps.tile([C, C], f32, name=f"tpc{b}", tag="p")
                nc.tensor.transpose(tpc_ps[:], ecr[:], ident[:])
                e_sum = mp.tile([C, C], f32, name=f"esum{b}")
                nc.vector.tensor_tensor(e_sum[:], e_main[:], ecr[:],
                                        op=mybir.AluOpType.add)
                nc.vector.tensor_tensor(e_cat[b][:], e_sum[:], tpc_ps[:],
                                        op=mybir.AluOpType.add)

            def emit_trigger(b):
                # the trigger "writes" a slice of e_cat (signals_writable)
                # so Tile orders it AFTER the real e_cat producer. A plain
                # gpsimd gate op cannot do this: the ucode-library placement
                # pass groups standard-lib ops after all remote-lib ops, so
                # a standard-lib gate gets sunk past the triggers.
                nc.gpsimd.trigger_dma(
                    count=None, signals_writable=[e_cat[b][0:1, 0:2]])

            def emit_reduce(b):
                # vector waits for all 7 peer partials (reg threshold = 14)
                nc.vector.wait_ge(rsem[b], vreg)
                acc = mp.tile([C, C], f32, name=f"ered{b}")
                nc.vector.tensor_tensor(acc[:], e_cat[b][:],
                                        eslot[b][0][:],
                                        op=mybir.AluOpType.add)
                for j in range(2, NCORES):
                    nc.vector.tensor_tensor(acc[:], acc[:],
                                            eslot[b][j - 1][:],
                                            op=mybir.AluOpType.add)
                e_red[b] = acc

            def emit_load_xq1():
                for k in range(NCHUNK):
                    nc.sync.dma_start(
                        xb16[1][k][:], xq1_d[:, k * CHUNK:(k + 1) * CHUNK])

            def emit_transpose_b0(k):
                # b0 hi chunk k -> [C, n] bf16 resident tiles for phase 2
                ht = hkeep[k]
                for g in range(TPC // 8):
                    tp = ps.tile([C, 8 * C], bf16,
                                 name=f"tp_0_{k}_{g}", tag="p")
                    for u in range(8):
                        j = g * 8 + u
                        nc.tensor.transpose(tp[:, u * C:(u + 1) * C],
                                            ht[:, j * C:(j + 1) * C],
                                            identb[:])
                    dst = xb16[0][k][:, g * 8 * C:(g + 1) * 8 * C]
                    if (k + g) % 2 == 0:
                        nc.vector.tensor_copy(dst, tp[:])
                    else:
                        nc.scalar.copy(dst, tp[:])

            def emit_softmax_pre(b):
                E_b = e_red[b][:]
                mcol = mp.tile([C, 1], f32, name=f"mcol{b}")
                nc.vector.tensor_reduce(mcol[:], E_b, axis=mybir.AxisListType.X,
                                        op=mybir.AluOpType.min)
                P_b = mp.tile([C, C], f32, name=f"P{b}")
                zcol = mp.tile([C, 1], f32, name=f"zcol{b}")
                # P = exp(min_row - E), zcol = rowsum(P); exponents <= 0.
                # P's diagonal is exp(min - ~+147000) == 0 exactly.
                nc.scalar.activation(P_b[:], E_b,
                                     mybir.ActivationFunctionType.Exp,
                                     bias=mcol[:], scale=-1.0,
                                     accum_out=zcol[:])
                rz = mp.tile([C, 1], f32, name=f"rz{b}")
                nc.vector.reciprocal(rz[:], zcol[:])
                scol = mp.tile([C, 1], f32, name=f"scol{b}")
                nc.vector.tensor_tensor(scol[:], rz[:], gcol[:],
                                        op=mybir.AluOpType.mult)
                # attn_s = (gamma/Z) * P + I  -> matmul computes x + gamma*attn@q
                nc.vector.tensor_scalar_mul(P_b[:], P_b[:], scol[:])
                nc.vector.tensor_add(P_b[:], P_b[:], ident[:])
                return P_b

            def emit_attnT(b, P_b, fin_copy_eng):
                tp2 = ps.tile([C, C], f32, name=f"tpP{b}", tag="p")
                nc.tensor.transpose(tp2[:], P_b[:], ident[:])
                attnT = mp.tile([C, C], bf16, name=f"attnT{b}")
                fin_copy_eng(attnT[:], tp2[:])  # fp32 psum -> bf16
                return attnT

            def emit_apply_chunk(b, attnT, k):
                ost = ostp.tile([C, CHUNK], f16, name=f"ost_{b}_{k}",
                                tag="ost")
                for h in range(CHUNK // (2 * OTILE)):
                    op = ps.tile([C, 2 * OTILE], f32, name=f"op_{b}_{k}_{h}",
                                 tag="p")
                    for u in range(2):
                        j = 2 * h + u
                        nc.tensor.matmul(
                            op[:, u * OTILE:(u + 1) * OTILE], attnT[:],
                            xb16[b][k][:, j * OTILE:(j + 1) * OTILE],
                            start=True, stop=True)
                    dst = ost[:, h * 2 * OTILE:(h + 1) * 2 * OTILE]
                    hh = k * (CHUNK // (2 * OTILE)) + h
                    if hh % 2 == 0:
                        nc.vector.tensor_copy(dst, op[:])
                    else:
                        nc.scalar.copy(dst, op[:])
                nc.sync.dma_start(o_d[b, :, k * CHUNK:(k + 1) * CHUNK],
                                  ost[:])
                return ost

            emit_preps(0)
            emit_phase1_mms(0)
            emit_trigger(0)
            emit_preps(1)
            emit_phase1_mms(1)
            emit_trigger(1)
            emit_load_xq1()
            # b0 reduction+softmax lands ~45us
            emit_reduce(0)
            P0 = emit_softmax_pre(0)
            attnT0 = emit_attnT(0, P0, nc.scalar.copy)
            # b1 reduction early in the vector queue (passes ~70us) so
            # attnT1 is ready well before the PE reaches phase 2 of b1
            emit_reduce(1)
            P1 = emit_softmax_pre(1)
            # b0: transpose chunk k then immediately apply it (PE
            # interleaved); output DMA starts streaming early
            for k in range(NCHUNK):
                emit_transpose_b0(k)
                emit_apply_chunk(0, attnT0, k)
            hkeep.clear()
            attnT1 = emit_attnT(1, P1, nc.scalar.copy)
            ost_last = None
            for k in range(NCHUNK):
                ost_last = emit_apply_chunk(1, attnT1, k)
            # keep gpsimd's body alive until the last output tile exists:
            # a standard-lib op here also swaps the gpsimd ucode library
            # back in hidden slack, so the end-of-body DGE drain is cheap
            # and never throttles the DMA engines mid-kernel.
            jend = mp.tile([C, 2], f16, name="jend")
            nc.gpsimd.tensor_tensor(jend[:], ost_last[:, 0:2],
                                    ost_last[:, 2:4],
                                    op=mybir.AluOpType.add)

    _log("tile context done; bacc compile start")
    nc.compile()
    _log("bacc compile done")
    return nc


def _get_nc():
    if "nc" not in _compiled:
        _compiled["nc"] = _build()
    return _compiled["nc"]


def kernel(x, gamma, _trace=False, _tmpdir=None):
    import ml_dtypes
    from concourse import bass_utils

    bf16 = ml_dtypes.bfloat16
    x = np.ascontiguousarray(np.asarray(x), dtype=np.float32)
    gamma = np.asarray(gamma, dtype=np.float32)
    q = x.reshape(B, C, N)
    hi = q.astype(bf16)
    lo = (q - hi.astype(np.float32)).astype(bf16)
    # tile-major transposed layout: A[r][b, p, t, c] = qT[b, r*NLOC+t*128+p, c]
    Ahi = np.ascontiguousarray(
        hi.reshape(B, C, NCORES, T, C).transpose(2, 0, 4, 3, 1)
    ).reshape(NCORES, B, C, T * C)
    Alo = np.ascontiguousarray(
        lo.reshape(B, C, NCORES, T, C).transpose(2, 0, 4, 3, 1)
    ).reshape(NCORES, B, C, T * C)
    gcol = np.full((C, 1), gamma[0], dtype=np.float32)
    ident = np.eye(C, dtype=np.float32)
    identb = np.eye(C, dtype=bf16)
    thresh = np.array([[1, 2 * (NCORES - 1)]], dtype=np.int32)

    in_maps = []
    for r in range(NCORES):
        in_maps.append({
            "qhT": Ahi[r],
            "qlT": Alo[r],
            "xq1": np.ascontiguousarray(
                hi[1, :, r * NLOC:(r + 1) * NLOC]),
            "gamma_col": gcol,
            "ident": ident,
            "identb": identb,
            "thresh": thresh,
        })

    nc = _get_nc()
    _log("launching run_bass_kernel_spmd")
    res = bass_utils.run_bass_kernel_spmd(
        nc, in_maps, core_ids=list(range(NCORES)), trace=_trace,
        tmpdir=_tmpdir)
    outs = [res.results[r]["out"] for r in range(NCORES)]
    full = np.concatenate(outs, axis=2).astype(np.float32)
    full = full.reshape(B, C, D, H, W)
    if _trace:
        return full.astype(np.float32, copy=False), res
    return full.astype(np.float32, copy=False)


# revision 12
# speedup vs baseline: 1.1683x; 1.1683x over previous
"""Channel-attention module kernel for 8 Trainium2 NeuronCores.

reference semantics (B=2, C=128, N=D*H*W=147456):
    q = x.reshape(B, C, N)
    energy = q @ q^T                  # [B, C, C]
    attn = softmax(rowmax(energy) - energy, axis=-1)
          = softmax(-energy, axis=-1)             (rowmax shift is a no-op)
    out = attn @ q
    return x + gamma * out

Sharding: sequence-parallel over N. Core r owns columns
[r*N/8, (r+1)*N/8) of q for both batches. Each core computes a partial
energy (contraction over its local n), the partials are summed across
cores, each core computes softmax and applies attention to its local
columns.

Scheme (v10.2) — no collectives at all. hw-trace findings that shaped it:
  - The CC-engine path (ncfw barrier + AllReduce) doesn't deliver the
    energy sum until ~113us. Instead each core broadcasts its partial
    [C,C] energy per batch to peer r^j's slot j over remote_dma_broadcast
    (XOR-relative dests, 7 one-dest bcasts per batch; slots j>=4 land
    permuted j^2 but the sum is permutation invariant). With no
    collective in the NEFF there is no ncfw cold-start barrier either:
    b0's summed energy is ready ~45us, b1's ~70us.
  - Receiver waits use REGISTER thresholds loaded from an input tensor
    (sim reads 0 -> no scheduler deadlock; HW reads 14), behind a
    TRACKED tensor_copy of the threshold tile (bare reg_load races the
    input DMA). Remote sems are cleared at kernel start on the engine
    that waits on them (sem state persists across NEFF loads).
  - Descriptor PREPS are emitted before phase 1 against pre-initialized
    e_cat tiles (descgen only captures addresses), so the reduce adds
    chain only on the remote sem, not on our own descgen. The TRIGGERS
    are gated on the real e_cat values via tiny gpsimd reads.
  - gpsimd's end-of-body DGE DRAIN costs ~50us once the remote_dma ucode
    library is loaded, and it throttles ALL DMA while it runs. A junk
    standard-lib gpsimd op gated on the last output tile (a) swaps the
    library back in hidden slack and (b) delays the drain past the last
    compute.
  - phase 2 per batch: out = attn_s @ q in bf16 (attn_s = gamma/Z*P + I
    folds the residual; P diagonal exactly 0). b0 uses PE-transposed
    tiles (transpose(k) interleaved with apply(k)); b1's operand xq1
    ([C,n] bf16, just a slice of hi) streams from HBM after all hi/lo
    chunks, where it contends with nothing.
  - hi/lo bf16 split energy as in v9 (two bf16 chains; rel err 1.7e-3
    vs the 2e-2 gate).
"""

import sys

sys.path.insert(0, "/opt/trn_rl_repo")

import numpy as np

B, C = 2, 128
D, H, W = 16, 96, 96
N = D * H * W  # 147456
NCORES = 8
NLOC = N // NCORES  # 18432
T = NLOC // C  # 144 n-tiles of 128 per batch
CHUNK = 2048
NCHUNK = NLOC // CHUNK  # 9
TPC = CHUNK // C  # 16 n-tiles per chunk
OTILE = 512

_compiled = {}


def _log(msg):
    import time as _t
    print(f"[kernel {_t.strftime('%H:%M:%S')}] {msg}", flush=True)


def _build():
    import concourse.bacc as bacc
    import concourse.tile as tile
    import concourse.mybir as mybir
    from concourse.tile_rust import add_dep_helper

    _log("build start")

    f32 = mybir.dt.float32
    f16 = mybir.dt.float16
    bf16 = mybir.dt.bfloat16
    i32 = mybir.dt.int32
    nc = bacc.Bacc("TRN2", target_bir_lowering=False, debug=False,
                   num_devices=NCORES)

    hi_d = nc.dram_tensor("qhT", [B, C, T * C], bf16, kind="ExternalInput").ap()
    lo_d = nc.dram_tensor("qlT", [B, C, T * C], bf16, kind="ExternalInput").ap()
    xq1_d = nc.dram_tensor("xq1", [C, NLOC], bf16, kind="ExternalInput").ap()
    g_d = nc.dram_tensor("gamma_col", [C, 1], f32, kind="ExternalInput").ap()
    id_d = nc.dram_tensor("ident", [C, C], f32, kind="ExternalInput").ap()
    idb_d = nc.dram_tensor("identb", [C, C], bf16, kind="ExternalInput").ap()
    th_d = nc.dram_tensor("thresh", [1, 2], i32, kind="ExternalInput").ap()
    o_d = nc.dram_tensor("out", [B, C, NLOC], f16, kind="ExternalOutput").ap()

    rsem = [nc.alloc_semaphore(f"rsem_e{b}") for b in range(B)]
    lsem = nc.alloc_semaphore("lsem_gather")

    with tile.TileContext(nc) as tc:
        with (
            tc.tile_pool(name="hring", bufs=NCHUNK + 4) as hp,
            tc.tile_pool(name="lring", bufs=4) as lp,
            tc.tile_pool(name="xb16", bufs=B * NCHUNK) as xbp,
            tc.tile_pool(name="eps", bufs=2, space="PSUM") as eps,
            tc.tile_pool(name="ps", bufs=3, space="PSUM") as ps,
            tc.tile_pool(name="misc", bufs=1) as mp,
            tc.tile_pool(name="ost", bufs=3) as ostp,
        ):
            # --- sem hygiene: clear on the engine that waits, first thing.
            # Receivers' clears run within ~1us of kernel start; the first
            # remote increments arrive only after some sender finishes its
            # b0 partial energy (~40us in), so clears always win the race.
            rclr = [nc.vector.sem_clear(rsem[b]) for b in range(B)]
            nc.gpsimd.sem_clear(lsem)

            # remote-sem wait threshold in a register (see module docstring)
            th = mp.tile([1, 2], i32, name="th_sb")
            nc.sync.dma_start(th[:], th_d[:])
            th2 = mp.tile([1, 2], i32, name="th2_sb")
            thcp = nc.vector.tensor_copy(th2[:], th[:])
            # pin the clears BEFORE the (early) th2 copy: a late-scheduled
            # clear would wipe peer increments that have already arrived.
            for b in range(B):
                add_dep_helper(thcp.ins, rclr[b].ins, sync=False,
                               reason="rsem clears run at kernel start")
            vreg = nc.vector.alloc_register("rth")
            vld = nc.vector.reg_load(vreg, th2[0:1, 1:2])

            ident = mp.tile([C, C], f32, name="ident_sb")
            identb = mp.tile([C, C], bf16, name="identb_sb")
            nc.sync.dma_start(identb[:], idb_d[:])
            nc.sync.dma_start(ident[:], id_d[:])
            # first chunk split so the PE starts during the DMA ramp
            ht0 = hp.tile([C, CHUNK], bf16, name="h_0_0", tag="h")
            nc.sync.dma_start(ht0[:, 0:512], hi_d[0, :, 0:512])
            lt0 = lp.tile([C, CHUNK], bf16, name="l_0_0", tag="l")
            nc.sync.dma_start(lt0[:, 0:512], lo_d[0, :, 0:512])
            nc.sync.dma_start(ht0[:, 512:CHUNK], hi_d[0, :, 512:CHUNK])
            nc.sync.dma_start(lt0[:, 512:CHUNK], lo_d[0, :, 512:CHUNK])
            gcol = mp.tile([C, 1], f32, name="gcol")
            nc.sync.dma_start(gcol[:], g_d[:])

            xb16 = [[xbp.tile([C, CHUNK], bf16, name=f"xb_{b}_{k}", tag="xb")
                     for k in range(NCHUNK)] for b in range(B)]

            # energy gather slots: slot j holds the partial of rank r^j
            # (j>=4 lands permuted j^2 -- sum is invariant)
            eslot = [[mp.tile([C, C], f32, name=f"esl_{b}_{j}")
                      for j in range(1, NCORES)] for b in range(B)]
            # e_cat: local partial per batch. Pre-initialized (copy of
            # ident, value irrelevant) so the descriptor preps emitted
            # before phase 1 have a tracked producer.
            e_cat = []
            for b in range(B):
                t_ = mp.tile([C, C], f32, name=f"e_cat{b}")
                nc.scalar.copy(t_[:], ident[:])
                e_cat.append(t_)
            e_red = [None, None]

            def emit_preps(b):
                # descriptor preps (addresses only; data is read at trigger
                # time by the DMA engines). Emitted before this batch's
                # phase-1 matmuls so descgen runs early on gpsimd.
                for j in range(1, NCORES):
                    rdests = [None] * 8
                    rdests[j] = (0, j)
                    nc.gpsimd.remote_dma_broadcast(
                        eslot[b][j - 1][:], e_cat[b][:], rsem[b], lsem,
                        rdests=rdests)

            hkeep = {}  # live hi chunks of batch 0 (for the transposes)

            def emit_phase1_mms(b):
                e_main = eps.tile([C, C], f32, name=f"em{b}", tag="e")
                e_cross = eps.tile([C, C], f32, name=f"ec{b}", tag="e")
                for k in range(NCHUNK):
                    if b == 0 and k == 0:
                        ht, lt = ht0, lt0
                    else:
                        ht = hp.tile([C, CHUNK], bf16, name=f"h_{b}_{k}",
                                     tag="h")
                        nc.sync.dma_start(
                            ht[:], hi_d[b, :, k * CHUNK:(k + 1) * CHUNK])
                        lt = lp.tile([C, CHUNK], bf16, name=f"l_{b}_{k}",
                                     tag="l")
                        nc.sync.dma_start(
                            lt[:], lo_d[b, :, k * CHUNK:(k + 1) * CHUNK])
                    if b == 0:
                        hkeep[k] = ht
                    if b == 0 and k == 0:
                        # consume the quarter-chunk first so the PE starts
                        # as early as possible during the DMA ramp
                        order = [("hh", j) for j in range(4)] \
                            + [("hl", j) for j in range(4)] \
                            + [p for j in range(4, TPC)
                               for p in (("hh", j), ("hl", j))]
                    else:
                        order = [p for j in range(TPC)
                                 for p in (("hh", j), ("hl", j))]
                    for kind, j in order:
                        t = k * TPC + j
                        hs = ht[:, j * C:(j + 1) * C]
                        if kind == "hh":
                            nc.tensor.matmul(e_main[:], hs, hs,
                                             start=(t == 0), stop=(t == T - 1))
                        else:
                            nc.tensor.matmul(e_cross[:], hs,
                                             lt[:, j * C:(j + 1) * C],
                                             start=(t == 0), stop=(t == T - 1))
                # E_partial = e_main + e_cross + e_cross^T
                ecr = mp.tile([C, C], f32, name=f"ecr{b}")
                nc.vector.tensor_copy(ecr[:], e_cross[:])
                tpc_ps = ps.tile([C, C], f32, name=f"tpc{b}", tag="p")
                nc.tensor.transpose(tpc_ps[:], ecr[:], ident[:])
                e_sum = mp.tile([C, C], f32, name=f"esum{b}")
                nc.vector.tensor_tensor(e_sum[:], e_main[:], ecr[:],
                                        op=mybir.AluOpType.add)
                return nc.vector.tensor_tensor(e_cat[b][:], e_sum[:],
                                                tpc_ps[:],
                                                op=mybir.AluOpType.add)

            def emit_trigger(b):
                # the trigger "writes" a slice of e_cat (signals_writable)
                # so Tile orders it AFTER the real e_cat producer. A plain
                # gpsimd gate op cannot do this: the ucode-library placement
                # pass groups standard-lib ops after all remote-lib ops, so
                # a standard-lib gate gets sunk past the triggers.
                nc.gpsimd.trigger_dma(
                    count=None, signals_writable=[e_cat[b][0:1, 0:2]])

            def emit_reduce(b, anchor):
                # vector waits for all 7 peer partials (reg threshold = 14).
                # The scheduler reorders within an engine and does NOT track
                # the wait's register read or the sem state, so pin the wait
                # explicitly: after reg_load, after the sem clear, and after
                # `anchor` (an instruction marking its intended queue slot
                # -- hoisting the wait earlier would block the vector queue
                # and deadlock all 8 cores). The first add is pinned after
                # the wait; the rest chain on acc.
                w = nc.vector.wait_ge(rsem[b], vreg)
                add_dep_helper(w.ins, vld.ins, sync=False,
                               reason="threshold register written before wait")
                add_dep_helper(w.ins, rclr[b].ins, sync=False,
                               reason="rsem cleared before wait")
                add_dep_helper(w.ins, anchor.ins, sync=False,
                               reason="wait sits at its emission slot")
                acc = mp.tile([C, C], f32, name=f"ered{b}")
                a1 = nc.vector.tensor_tensor(acc[:], e_cat[b][:],
                                             eslot[b][0][:],
                                             op=mybir.AluOpType.add)
                add_dep_helper(a1.ins, w.ins, sync=False,
                               reason="slot reads gated by rsem wait")
                for j in range(2, NCORES):
                    nc.vector.tensor_tensor(acc[:], acc[:],
                                            eslot[b][j - 1][:],
                                            op=mybir.AluOpType.add)
                e_red[b] = acc

            def emit_load_xq1():
                for k in range(NCHUNK):
                    nc.sync.dma_start(
                        xb16[1][k][:], xq1_d[:, k * CHUNK:(k + 1) * CHUNK])

            def emit_transpose_b0(k):
                # b0 hi chunk k -> [C, n] bf16 resident tiles for phase 2
                ht = hkeep[k]
                last = None
                for g in range(TPC // 8):
                    tp = ps.tile([C, 8 * C], bf16,
                                 name=f"tp_0_{k}_{g}", tag="p")
                    for u in range(8):
                        j = g * 8 + u
                        nc.tensor.transpose(tp[:, u * C:(u + 1) * C],
                                            ht[:, j * C:(j + 1) * C],
                                            identb[:])
                    dst = xb16[0][k][:, g * 8 * C:(g + 1) * 8 * C]
                    if (k + g) % 2 == 0:
                        last = nc.vector.tensor_copy(dst, tp[:])
                    else:
                        nc.scalar.copy(dst, tp[:])
                return last

            def emit_softmax_pre(b):
                E_b = e_red[b][:]
                mcol = mp.tile([C, 1], f32, name=f"mcol{b}")
                nc.vector.tensor_reduce(mcol[:], E_b, axis=mybir.AxisListType.X,
                                        op=mybir.AluOpType.min)
                P_b = mp.tile([C, C], f32, name=f"P{b}")
                zcol = mp.tile([C, 1], f32, name=f"zcol{b}")
                # P = exp(min_row - E), zcol = rowsum(P); exponents <= 0.
                # P's diagonal is exp(min - ~+147000) == 0 exactly.
                nc.scalar.activation(P_b[:], E_b,
                                     mybir.ActivationFunctionType.Exp,
                                     bias=mcol[:], scale=-1.0,
                                     accum_out=zcol[:])
                rz = mp.tile([C, 1], f32, name=f"rz{b}")
                nc.vector.reciprocal(rz[:], zcol[:])
                scol = mp.tile([C, 1], f32, name=f"scol{b}")
                nc.vector.tensor_tensor(scol[:], rz[:], gcol[:],
                                        op=mybir.AluOpType.mult)
                # attn_s = (gamma/Z) * P + I  -> matmul computes x + gamma*attn@q
                nc.vector.tensor_scalar_mul(P_b[:], P_b[:], scol[:])
                nc.vector.tensor_add(P_b[:], P_b[:], ident[:])
                return P_b

            def emit_attnT(b, P_b, fin_copy_eng):
                tp2 = ps.tile([C, C], f32, name=f"tpP{b}", tag="p")
                nc.tensor.transpose(tp2[:], P_b[:], ident[:])
                attnT = mp.tile([C, C], bf16, name=f"attnT{b}")
                fin_copy_eng(attnT[:], tp2[:])  # fp32 psum -> bf16
                return attnT

            def emit_apply_chunk(b, attnT, k):
                ost = ostp.tile([C, CHUNK], f16, name=f"ost_{b}_{k}",
                                tag="ost")
                for h in range(CHUNK // (2 * OTILE)):
                    op = ps.tile([C, 2 * OTILE], f32, name=f"op_{b}_{k}_{h}",
                                 tag="p")
                    for u in range(2):
                        j = 2 * h + u
                        nc.tensor.matmul(
                            op[:, u * OTILE:(u + 1) * OTILE], attnT[:],
                            xb16[b][k][:, j * OTILE:(j + 1) * OTILE],
                            start=True, stop=True)
                    dst = ost[:, h * 2 * OTILE:(h + 1) * 2 * OTILE]
                    hh = k * (CHUNK // (2 * OTILE)) + h
                    if hh % 2 == 0:
                        nc.vector.tensor_copy(dst, op[:])
                    else:
                        nc.scalar.copy(dst, op[:])
                nc.sync.dma_start(o_d[b, :, k * CHUNK:(k + 1) * CHUNK],
                                  ost[:])
                return ost

            emit_preps(0)
            ecat0_add = emit_phase1_mms(0)
            emit_trigger(0)
            # b0 reduction+softmax lands ~45us, anchored after e_cat0's add
            emit_reduce(0, ecat0_add)
            P0 = emit_softmax_pre(0)
            emit_preps(1)
            ecat1_add = emit_phase1_mms(1)
            emit_trigger(1)
            emit_load_xq1()
            attnT0 = emit_attnT(0, P0, nc.scalar.copy)
            # b0: transpose chunk k then immediately apply it (PE
            # interleaved); output DMA starts streaming early
            last_cp = None
            for k in range(NCHUNK):
                last_cp = emit_transpose_b0(k) or last_cp
                emit_apply_chunk(0, attnT0, k)
            hkeep.clear()
            # b1 reduction sits after the apply-0 copy stream in the vector
            # queue (rsem1 passed long ago; no queue blocking)
            emit_reduce(1, last_cp)
            P1 = emit_softmax_pre(1)
            attnT1 = emit_attnT(1, P1, nc.scalar.copy)
            ost_last = None
            for k in range(NCHUNK):
                ost_last = emit_apply_chunk(1, attnT1, k)
            # keep gpsimd's body alive until the last output tile exists:
            # a standard-lib op here also swaps the gpsimd ucode library
            # back in hidden slack, so the end-of-body DGE drain is cheap
            # and never throttles the DMA engines mid-kernel.
            jend = mp.tile([C, 2], f16, name="jend")
            nc.gpsimd.tensor_tensor(jend[:], ost_last[:, 0:2],
                                    ost_last[:, 2:4],
                                    op=mybir.AluOpType.add)

    _log("tile context done; bacc compile start")
    nc.compile()
    _log("bacc compile done")
    return nc


def _get_nc():
    if "nc" not in _compiled:
        _compiled["nc"] = _build()
    return _compiled["nc"]


def kernel(x, gamma, _trace=False, _tmpdir=None):
    import ml_dtypes
    from concourse import bass_utils

    bf16 = ml_dtypes.bfloat16
    x = np.ascontiguousarray(np.asarray(x), dtype=np.float32)
    gamma = np.asarray(gamma, dtype=np.float32)
    q = x.reshape(B, C, N)
    hi = q.astype(bf16)
    lo = (q - hi.astype(np.float32)).astype(bf16)
    # tile-major transposed layout: A[r][b, p, t, c] = qT[b, r*NLOC+t*128+p, c]
    Ahi = np.ascontiguousarray(
        hi.reshape(B, C, NCORES, T, C).transpose(2, 0, 4, 3, 1)
    ).reshape(NCORES, B, C, T * C)
    Alo = np.ascontiguousarray(
        lo.reshape(B, C, NCORES, T, C).transpose(2, 0, 4, 3, 1)
    ).reshape(NCORES, B, C, T * C)
    gcol = np.full((C, 1), gamma[0], dtype=np.float32)
    ident = np.eye(C, dtype=np.float32)
    identb = np.eye(C, dtype=bf16)
    thresh = np.array([[1, 2 * (NCORES - 1)]], dtype=np.int32)

    in_maps = []
    for r in range(NCORES):
        in_maps.append({
            "qhT": Ahi[r],
            "qlT": Alo[r],
            "xq1": np.ascontiguousarray(
                hi[1, :, r * NLOC:(r + 1) * NLOC]),
            "gamma_col": gcol,
            "ident": ident,
            "identb": identb,
            "thresh": thresh,
        })

    nc = _get_nc()
    _log("launching run_bass_kernel_spmd")
    res = bass_utils.run_bass_kernel_spmd(
        nc, in_maps, core_ids=list(range(NCORES)), trace=_trace,
        tmpdir=_tmpdir)
    outs = [res.results[r]["out"] for r in range(NCORES)]
    full = np.concatenate(outs, axis=2).astype(np.float32)
    full = full.reshape(B, C, D, H, W)
    if _trace:
        return full.astype(np.float32, copy=False), res
    return full.astype(np.float32, copy=False)
